# revision 42
# baseline (speedup 1.0000x reference)
"""Causal multi-head attention on 8 TRN2 NeuronCores.

Sharding: core c -> (batch b = c // 2, head-half hh = c % 2).
Each core computes QKV for its 8 heads over the full sequence of its batch,
causal flash attention, and a partial out-projection using its 512 rows of
w_out. The host sums the two partials per batch (the "all-reduce" of the
tensor-parallel out projection).

v3 layout (all matmul operands bf16):
  xs[d]    [128, 2048] whole input row-block, loaded once
  KT[c][j] [128, 512]  K^T head pair j, token chunk c (head 2j rows 0:64,
                       head 2j+1 rows 64:128)
  V[t]     [128, 584]  V token-tile t, 8 heads x (64 cols + ones col),
                       padded so AV can load 128-col weight slices (FWL)
  QT[c][j] [128, 512]  Q^T (even head rows 0:64, odd rows 64:128)

Per k-block pair, scores for both heads land in ONE 4-bank PSUM tile
  s_all = [a-even | b-even | a-odd | b-odd]  (4 x 512 cols)
so a single wide EXP covers the pair and all four S matmuls become ready
together; emitted as (a-even,a-odd),(b-even,b-odd) they run pairwise
CONCURRENTLY on the PE via K=64 row tiling (tile_position (0,0)/(64,0)).

AV uses lhsT = V[pk][:, 65*h : 65*h+128] (128-col weight loads -> FWL);
output rows 65:127 are garbage that lands in unread PSUM partitions.

Schedule: QKV projection for chunk c+1 and the out-projection for chunk
c-1 are woven into the attention pair loop of chunk c.

Shapes (hardcoded): B=4, T=2048, D=1024, H=16, HD=64.
"""
import sys

for _p in ('/opt/trn_rl_repo', '/root/.axon_site/_ro/trn_rl_repo'):
    if _p not in sys.path:
        sys.path.insert(0, _p)

import numpy as np

B, T, D = 4, 2048, 1024
H, HD = 16, 64
HPC = H // 2          # heads per core = 8
DPC = HPC * HD        # out-dims per core = 512
N_CORES = 8

_nc_cache = {}


def _build_nc():
    import concourse.bacc as bacc
    import concourse.mybir as mybir
    from concourse.tile import TileContext

    F32 = mybir.dt.float32
    BF16 = mybir.dt.bfloat16
    AF = mybir.ActivationFunctionType
    ALU = mybir.AluOpType

    CH = 512              # token chunk (both proj and attention q-chunk)
    NKB = T // 128        # 16 k-blocks
    NC = T // CH          # 4 chunks
    NDT = D // 128        # 8 input-dim tiles
    VW = HPC * (HD + 1) + 64   # V tile width = 584 (64-col pad for LDW)

    nc = bacc.Bacc('TRN2', target_bir_lowering=False, debug=False)
    xT_d = nc.dram_tensor('xT', [D, T], BF16, kind='ExternalInput')
    wq_d = nc.dram_tensor('wq', [D, DPC], BF16, kind='ExternalInput')
    wk_d = nc.dram_tensor('wk', [D, DPC], BF16, kind='ExternalInput')
    wv_d = nc.dram_tensor('wv', [D, DPC], BF16, kind='ExternalInput')
    wo_d = nc.dram_tensor('wo', [DPC, D], BF16, kind='ExternalInput')
    po_d = nc.dram_tensor('po', [T, D], BF16, kind='ExternalOutput')

    with nc.allow_low_precision(reason='bf16 matmuls by design'), \
            TileContext(nc) as tc:
        with (
            tc.tile_pool(name='w', bufs=1) as w_pool,
            tc.tile_pool(name='kt', bufs=1) as kt_pool,
            tc.tile_pool(name='vv', bufs=1) as v_pool,
            tc.tile_pool(name='xs', bufs=1) as x_pool,
            tc.tile_pool(name='qt', bufs=2) as qt_pool,
            tc.tile_pool(name='pt', bufs=4) as pt_pool,
            tc.tile_pool(name='ao', bufs=3) as ao_pool,
            tc.tile_pool(name='osb', bufs=2) as osb_pool,
            tc.tile_pool(name='small', bufs=2) as sm_pool,
            tc.tile_pool(name='ps_s', bufs=2, space='PSUM') as ps_s,
            tc.tile_pool(name='ps_ot', bufs=4, space='PSUM') as ps_ot,
        ):
            WK = [w_pool.tile([128, DPC], BF16, tag=f'wk{d}', name=f'wks{d}')
                  for d in range(NDT)]
            WV = [w_pool.tile([128, DPC], BF16, tag=f'wv{d}', name=f'wvs{d}')
                  for d in range(NDT)]
            WQ = [w_pool.tile([128, DPC], BF16, tag=f'wq{d}', name=f'wqs{d}')
                  for d in range(NDT)]
            WO = [w_pool.tile([128, D], BF16, tag=f'wo{d}', name=f'wos{d}')
                  for d in range(4)]
            XS = [x_pool.tile([128, T], BF16, tag=f'x{d}', name=f'xs{d}')
                  for d in range(NDT)]
            KT = [[kt_pool.tile([128, CH], BF16, tag=f'kt{c}_{j}',
                                name=f'kt{c}_{j}') for j in range(4)]
                  for c in range(NC)]
            V = [v_pool.tile([128, VW], BF16, tag=f'v{t}', name=f'v{t}')
                 for t in range(NKB)]

            # pre-warm the ACT exp table and the gpsimd library so the
            # first real exp / affine_select doesn't pay the load
            warm = sm_pool.tile([1, 16], F32, tag='warm', bufs=1)
            warm2 = sm_pool.tile([2, 16], F32, tag='warm2', bufs=1)
            nc.vector.memset(warm[:, :], 0.0)
            nc.scalar.activation(warm[:, :], warm[:, :], AF.Exp)
            nc.gpsimd.affine_select(
                out=warm[:, :], in_=warm[:, :], compare_op=ALU.is_ge,
                fill=0.0, base=0, channel_multiplier=-1, pattern=[[1, 16]])
            nc.gpsimd.partition_broadcast(warm2[:, :], warm[:, :])

            # DMAs ordered by first use: WK and the first x chunk feed the
            # first proj group; WQ next so attention(0,0) S/exp work can
            # fill the PE while the V projection still waits on WV
            for d in range(NDT):
                nc.sync.dma_start(WK[d][:, :], wk_d[d*128:(d+1)*128, :])
                nc.sync.dma_start(XS[d][:, 0:CH],
                                  xT_d[d*128:(d+1)*128, 0:CH])
            for d in range(NDT):
                nc.sync.dma_start(WQ[d][:, :], wq_d[d*128:(d+1)*128, :])
            for d in range(NDT):
                nc.sync.dma_start(XS[d][:, CH:T//2],
                                  xT_d[d*128:(d+1)*128, CH:T//2])
            for d in range(NDT):
                nc.sync.dma_start(WV[d][:, :], wv_d[d*128:(d+1)*128, :])
            for d in range(NDT):
                nc.sync.dma_start(XS[d][:, T//2:T],
                                  xT_d[d*128:(d+1)*128, T//2:T])
            for d in range(4):
                nc.sync.dma_start(WO[d][:, :], wo_d[d*128:(d+1)*128, :])
            # ones columns for the softmax-denominator trick + zero pad
            for t in range(NKB):
                vt3 = V[t][:, 0:HPC*(HD+1)].rearrange(
                    'p (h c) -> p h c', c=HD + 1)
                nc.gpsimd.memset(vt3[:, :, HD], 1.0)
                nc.gpsimd.memset(V[t][:, HPC*(HD+1):VW], 0.0)

            qt_tiles = {}
            ao_tiles = {}

            def proj_group(kind, c, i):
                """One 8-matmul projection group for token chunk c."""
                tok = slice(c*CH, (c+1)*CH)
                if kind == 'K':     # KT[c][i]: out [128 dout, CH tok]
                    pp = ps_ot.tile([128, CH], F32, tag='ot', name='pp')
                    for d in range(NDT):
                        nc.tensor.matmul(
                            pp[:, :], lhsT=WK[d][:, i*128:(i+1)*128],
                            rhs=XS[d][:, tok],
                            start=(d == 0), stop=(d == NDT - 1))
                    nc.vector.tensor_copy(KT[c][i][:, :], pp[:, :])
                elif kind == 'V':   # V block c*4+i: out [128 tok, DPC dout]
                    pv = ps_ot.tile([128, DPC], F32, tag='ot', name='pv')
                    for d in range(NDT):
                        nc.tensor.matmul(
                            pv[:, :],
                            lhsT=XS[d][:, c*CH+i*128:c*CH+(i+1)*128],
                            rhs=WV[d][:, :],
                            start=(d == 0), stop=(d == NDT - 1))
                    vt3 = V[c*4 + i][:, 0:HPC*(HD+1)].rearrange(
                        'p (h c) -> p h c', c=HD + 1)
                    nc.vector.tensor_copy(
                        vt3[:, :, 0:HD],
                        pv.rearrange('p (h c) -> p h c', c=HD))
                elif kind == 'Q':   # QT[c][i]: out [128 dout, CH tok]
                    pq = ps_ot.tile([128, CH], F32, tag='ot', name='pq')
                    for d in range(NDT):
                        nc.tensor.matmul(
                            pq[:, :], lhsT=WQ[d][:, i*128:(i+1)*128],
                            rhs=XS[d][:, tok],
                            start=(d == 0), stop=(d == NDT - 1))
                    qt = qt_pool.tile([128, CH], BF16, tag=f'qt{i}',
                                      name=f'qt{c}_{i}')
                    nc.vector.tensor_copy(qt[:, :], pq[:, :])
                    qt_tiles.setdefault(c, {})[i] = qt

            def outproj_qt(c, qt_i, pj_pool=None, pj_tag='ot'):
                """Out-projection for query rows [c*CH + qt_i*128 ...)."""
                ao = ao_tiles[c]
                q0 = c * CH
                os = osb_pool.tile([128, D], BF16, tag='os', name='os')
                pool = pj_pool or ps_ot
                pjs = [pool.tile([128, 512], F32, tag=pj_tag, name='pj')
                       for _ in range(2)]
                # d outer so each ao weight load serves both halves
                for d in range(4):
                    for half in range(2):
                        nc.tensor.matmul(
                            pjs[half][:, :],
                            lhsT=ao[d][:, qt_i*128:(qt_i+1)*128],
                            rhs=WO[d][:, half*512:(half+1)*512],
                            start=(d == 0), stop=(d == 3))
                for half in range(2):
                    nc.vector.tensor_copy(
                        os[:, half*512:(half+1)*512], pjs[half][:, :])
                nc.sync.dma_start(
                    po_d[q0+qt_i*128:q0+(qt_i+1)*128, :], os[:, :])

            def attention_j(c, j, weave_items=(), tail_items=(), front=False):
                """Causal attention for head pair j over query chunk c.

                weave_items: filler proj/outproj groups emitted between
                k-blocks so TensorE has work during each exp shadow.
                tail_items: outproj groups emitted after the block loop on
                the freed 's' PSUM slots (chunk 3 tail overlap).
                """
                q0 = c * CH
                nkb = (q0 + CH) // 128
                QTj = qt_tiles[c][j]
                h0, h1 = 2*j, 2*j + 1
                ot0 = ps_ot.tile([128, CH], F32, tag='ot', name='ot0')
                ot1 = ps_ot.tile([128, CH], F32, tag='ot', name='ot1')
                witems = list(weave_items)
                boundary = (witems.pop() if len(witems) > 1 and not front
                            else None)
                npair = nkb // 2
                if witems:
                    window = (npair // 2) if front else npair
                    spacing = max(1, window // len(witems))
                else:
                    spacing = 0

                def emit_w(item):
                    kind, wc, wi = item
                    if kind == 'O':
                        outproj_qt(wc, wi)
                    else:
                        proj_group(kind, wc, wi)

                def emit_av(pk, pl, ppt, stop):
                    nc.tensor.matmul(
                        ot0[:, pl:CH],
                        lhsT=V[pk][:, 65*h0:65*h0+128],
                        rhs=ppt[:, pl:CH],
                        start=(pk == 0), stop=stop)
                    nc.tensor.matmul(
                        ot1[:, pl:CH],
                        lhsT=V[pk][:, 65*h1:65*h1+128],
                        rhs=ppt[:, CH+pl:2*CH],
                        start=(pk == 0), stop=stop)

                pend = []
                for kk in range(nkb // 2):
                    # batch S and AV in two-block runs: fewer exposed
                    # LDWEIGHTS at the S<->AV transitions
                    blocks = []
                    for k in (2*kk, 2*kk + 1):
                        lo = max(0, k*128 - q0)
                        if k == 0:
                            # first block of the pair loop: score into two
                            # transient 'ot' banks so this j's start never
                            # waits on the previous j's exp pipeline (the
                            # 's' slots recycle at ScalarE pace)
                            s = (ps_ot.tile([128, CH], F32, tag='ot',
                                            name='se'),
                                 ps_ot.tile([128, CH], F32, tag='ot',
                                            name='so'))
                            s_even, s_odd = (s[0][:, 0:CH], s[1][:, 0:CH])
                        else:
                            st = ps_s.tile([128, 2*CH], F32, tag='s',
                                           name='s')
                            s = st
                            s_even = st[:, lo:CH]
                            s_odd = st[:, CH+lo:2*CH]
                        pt = pt_pool.tile([128, 2*CH], BF16, tag='pt',
                                          name='pt')
                        ks = KT[k//4][j][:, (k % 4)*128:((k % 4)+1)*128]
                        # S for both heads, concurrent K=64 row tiles
                        nc.tensor.matmul(
                            s_even, lhsT=ks[0:64, :],
                            rhs=QTj[0:64, lo:CH], start=True, stop=True)
                        nc.tensor.matmul(
                            s_odd, lhsT=ks[64:128, :],
                            rhs=QTj[64:128, lo:CH], start=True, stop=True)
                        blocks.append((k, lo, s, pt))
                    for pk, pl, ppt in pend:
                        emit_av(pk, pl, ppt, stop=False)
                    pend = []
                    for k, lo, s, pt in blocks:
                        if isinstance(s, tuple):
                            nc.scalar.activation(
                                pt[:, 0:CH], s[0][:, 0:CH], AF.Exp)
                            nc.scalar.activation(
                                pt[:, CH:2*CH], s[1][:, 0:CH], AF.Exp)
                        elif lo:
                            # two-piece AP skips the unwritten columns
                            # [CH:CH+lo] between the head sections
                            s3 = s.rearrange('p (h c) -> p h c', h=2)
                            pt3 = pt.rearrange('p (h c) -> p h c', h=2)
                            nc.scalar.activation(
                                pt3[:, :, lo:CH], s3[:, :, lo:CH], AF.Exp)
                        else:
                            nc.scalar.activation(
                                pt[:, 0:2*CH], s[:, 0:2*CH], AF.Exp)
                        if k*128 >= q0:   # causal mask on diagonal block
                            for c0 in (0, CH):
                                nc.gpsimd.affine_select(
                                    out=pt[:, c0+lo:c0+lo+128],
                                    in_=pt[:, c0+lo:c0+lo+128],
                                    compare_op=ALU.is_ge, fill=0.0,
                                    base=0, channel_multiplier=-1,
                                    pattern=[[1, 128]])
                        pend.append((k, lo, pt))
                    if witems and kk % spacing == spacing - 1:
                        emit_w(witems.pop(0))
                for i, (pk, pl, ppt) in enumerate(pend):
                    emit_av(pk, pl, ppt, stop=(pk == nkb - 1))
                for item in witems:   # leftovers
                    emit_w(item)
                # normalize both heads of the pair
                rp0 = sm_pool.tile([1, CH], F32, tag='rp0', bufs=2)
                rp1 = sm_pool.tile([1, CH], F32, tag='rp1', bufs=2)
                din0 = sm_pool.tile([1, CH], F32, tag='din0', bufs=2)
                din1 = sm_pool.tile([1, CH], F32, tag='din1', bufs=2)
                nc.vector.tensor_copy(din0[:, :], ot0[HD:HD+1, :])
                nc.vector.tensor_copy(din1[:, :], ot1[HD:HD+1, :])
                nc.vector.reciprocal_approx_fast(out=rp0[:, :], in_=din0[:, :])
                nc.vector.reciprocal_approx_fast(out=rp1[:, :], in_=din1[:, :])
                rbs0 = sm_pool.tile([HD, CH], F32, tag='rbs0', bufs=2)
                rbs1 = sm_pool.tile([HD, CH], F32, tag='rbs1', bufs=2)
                nc.gpsimd.partition_broadcast(rbs0[:, :], rp0[:, :])
                nc.gpsimd.partition_broadcast(rbs1[:, :], rp1[:, :])
                ao = ao_tiles[c][j]
                nc.vector.tensor_tensor(
                    out=ao[0:HD, :], in0=ot0[0:HD, :], in1=rbs0[:, :],
                    op=ALU.mult)
                nc.vector.tensor_tensor(
                    out=ao[HD:128, :], in0=ot1[0:HD, :], in1=rbs1[:, :],
                    op=ALU.mult)
                if boundary is not None:
                    emit_w(boundary)
                for kind, wc, wi in tail_items:
                    outproj_qt(wc, wi, pj_pool=ps_s, pj_tag='s')

            # ---------------- emission schedule ----------------
            # chunk-0 projections run d-major across 4 concurrent PSUM
            # accumulators so the PE tracks the weight/x DMA arrival
            # instead of waiting for the full transfer per group
            def proj_chunk0(kind):
                pps = [ps_ot.tile([128, CH], F32, tag='ot', name=f'p0{i}')
                       for i in range(4)]
                for d in range(NDT):
                    for i in range(4):
                        if kind == 'K':
                            nc.tensor.matmul(
                                pps[i][:, :],
                                lhsT=WK[d][:, i*128:(i+1)*128],
                                rhs=XS[d][:, 0:CH],
                                start=(d == 0), stop=(d == NDT - 1))
                        elif kind == 'V':
                            nc.tensor.matmul(
                                pps[i][:, :],
                                lhsT=XS[d][:, i*128:(i+1)*128],
                                rhs=WV[d][:, :],
                                start=(d == 0), stop=(d == NDT - 1))
                        else:
                            nc.tensor.matmul(
                                pps[i][:, :],
                                lhsT=WQ[d][:, i*128:(i+1)*128],
                                rhs=XS[d][:, 0:CH],
                                start=(d == 0), stop=(d == NDT - 1))
                for i in range(4):
                    if kind == 'K':
                        nc.vector.tensor_copy(KT[0][i][:, :], pps[i][:, :])
                    elif kind == 'V':
                        vt3 = V[i][:, 0:HPC*(HD+1)].rearrange(
                            'p (h c) -> p h c', c=HD + 1)
                        nc.vector.tensor_copy(
                            vt3[:, :, 0:HD],
                            pps[i].rearrange('p (h c) -> p h c', c=HD))
                    else:
                        qt = qt_pool.tile([128, CH], BF16, tag=f'qt{i}',
                                          name=f'qt0_{i}')
                        nc.vector.tensor_copy(qt[:, :], pps[i][:, :])
                        qt_tiles.setdefault(0, {})[i] = qt

            proj_chunk0('K')
            proj_chunk0('Q')
            proj_chunk0('V')

            # per chunk c: attention(c) woven with filler matmul groups.
            # proj for chunk c+1 goes into chunk c, except V(3) which lands
            # in chunk 3's first head pair (chunk 3 is exp-bound, chunk 2
            # is matmul-bound); outproj(c) goes into chunk c+1; outproj(3)
            # trails chunk 3's last head pair on the freed 's' PSUM slots.
            for c in range(NC):
                ao_tiles[c] = [ao_pool.tile([128, CH], BF16, tag=f'ao{j}',
                                            name=f'ao{c}_{j}')
                               for j in range(4)]
                # round-robin the filler kinds; out-projection filler lands
                # two chunks later, where the schedule is exp-bound rather
                # than matmul-bound
                kinds = []
                if c + 1 < NC:
                    kinds.append([('K', c+1, i) for i in range(4)])
                    if c + 1 < NC - 1:
                        kinds.append([('V', c+1, i) for i in range(4)])
                    kinds.append([('Q', c+1, i) for i in range(4)])
                if c == 2:
                    kinds.append([('O', 0, i) for i in range(4)])
                    kinds.append([('O', 1, i) for i in range(4)])
                weave = [it for grp in zip(*kinds) for it in grp] if kinds \
                    else []
                if c == NC - 1:
                    # NOTE: ('O', c, 0) must be LAST in jweave[3]: it reads
                    # this chunk's ao[3] and may only be emitted after the
                    # final norm (the boundary slot), never in-loop.
                    jweave = {0: [('V', c, i) for i in range(4)],
                              1: [('O', c-1, 0), ('O', c-1, 1)],
                              2: [('O', c-1, 2)],
                              3: [('O', c-1, 3), ('O', c, 0)]}
                    for j in range(4):
                        attention_j(c, j, jweave[j], front=(j == 0),
                                    tail_items=([('O', c, i)
                                                 for i in range(1, 4)]
                                                if j == 3 else ()))
                else:
                    per_j = (len(weave) + 3) // 4
                    for j in range(4):
                        attention_j(c, j, weave[j*per_j:(j+1)*per_j])

    nc.compile()
    return nc


def _get_nc():
    if 'nc' not in _nc_cache:
        _nc_cache['nc'] = _build_nc()
    return _nc_cache['nc']


def kernel(x, w_qkv, w_out, _profile=False):
    import ml_dtypes
    from concourse.bass_utils import run_bass_kernel_spmd

    x = np.asarray(x, dtype=np.float32)
    w_qkv = np.asarray(w_qkv, dtype=np.float32)
    w_out = np.asarray(w_out, dtype=np.float32)

    nc = _get_nc()

    bf16 = ml_dtypes.bfloat16
    scale = np.float32(1.0 / np.sqrt(HD))
    in_maps = []
    for c in range(N_CORES):
        b, hh = c // 2, c % 2
        s, e = hh * DPC, (hh + 1) * DPC
        in_maps.append({
            'xT': np.ascontiguousarray(x[b].T).astype(bf16),
            'wq': np.ascontiguousarray(w_qkv[:, s:e] * scale).astype(bf16),
            'wk': np.ascontiguousarray(w_qkv[:, D+s:D+e]).astype(bf16),
            'wv': np.ascontiguousarray(w_qkv[:, 2*D+s:2*D+e]).astype(bf16),
            'wo': np.ascontiguousarray(w_out[s:e, :]).astype(bf16),
        })

    res = run_bass_kernel_spmd(nc, in_maps, core_ids=list(range(N_CORES)),
                               trace=_profile)
    out = np.empty((B, T, D), np.float32)
    for b in range(B):
        out[b] = (res.results[2*b]['po'].astype(np.float32)
                  + res.results[2*b+1]['po'].astype(np.float32))
    if _profile:
        return out, res
    return out


# revision 43
# speedup vs baseline: 1.0830x; 1.0830x over previous
"""Causal multi-head attention on 8 TRN2 NeuronCores.

Sharding: core c -> (batch b = c // 2, head-half hh = c % 2).
Each core computes QKV for its 8 heads over the full sequence of its batch,
causal flash attention, and a partial out-projection using its 512 rows of
w_out. The host sums the two partials per batch (the "all-reduce" of the
tensor-parallel out projection).

v3 layout (all matmul operands bf16):
  xs[d]    [128, 2048] whole input row-block, loaded once
  KT[c][j] [128, 512]  K^T head pair j, token chunk c (head 2j rows 0:64,
                       head 2j+1 rows 64:128)
  V[t]     [128, 584]  V token-tile t, 8 heads x (64 cols + ones col),
                       padded so AV can load 128-col weight slices (FWL)
  QT[c][j] [128, 512]  Q^T (even head rows 0:64, odd rows 64:128)

Per k-block pair, scores for both heads land in ONE 4-bank PSUM tile
  s_all = [a-even | b-even | a-odd | b-odd]  (4 x 512 cols)
so a single wide EXP covers the pair and all four S matmuls become ready
together; emitted as (a-even,a-odd),(b-even,b-odd) they run pairwise
CONCURRENTLY on the PE via K=64 row tiling (tile_position (0,0)/(64,0)).

AV uses lhsT = V[pk][:, 65*h : 65*h+128] (128-col weight loads -> FWL);
output rows 65:127 are garbage that lands in unread PSUM partitions.

Schedule: QKV projection for chunk c+1 and the out-projection for chunk
c-1 are woven into the attention pair loop of chunk c.

Shapes (hardcoded): B=4, T=2048, D=1024, H=16, HD=64.
"""
import sys

for _p in ('/opt/trn_rl_repo', '/root/.axon_site/_ro/trn_rl_repo'):
    if _p not in sys.path:
        sys.path.insert(0, _p)

import numpy as np

B, T, D = 4, 2048, 1024
H, HD = 16, 64
HPC = H // 2          # heads per core = 8
DPC = HPC * HD        # out-dims per core = 512
N_CORES = 8

_nc_cache = {}


def _build_nc():
    import concourse.bacc as bacc
    import concourse.mybir as mybir
    from concourse.tile import TileContext

    F32 = mybir.dt.float32
    BF16 = mybir.dt.bfloat16
    AF = mybir.ActivationFunctionType
    ALU = mybir.AluOpType

    CH = 512              # token chunk (both proj and attention q-chunk)
    NKB = T // 128        # 16 k-blocks
    NC = T // CH          # 4 chunks
    NDT = D // 128        # 8 input-dim tiles
    VW = HPC * (HD + 1) + 64   # V tile width = 584 (64-col pad for LDW)

    nc = bacc.Bacc('TRN2', target_bir_lowering=False, debug=False)
    xT_d = nc.dram_tensor('xT', [D, T], BF16, kind='ExternalInput')
    wq_d = nc.dram_tensor('wq', [D, DPC], BF16, kind='ExternalInput')
    wk_d = nc.dram_tensor('wk', [D, DPC], BF16, kind='ExternalInput')
    wv_d = nc.dram_tensor('wv', [D, DPC], BF16, kind='ExternalInput')
    wo_d = nc.dram_tensor('wo', [DPC, D], BF16, kind='ExternalInput')
    po_d = nc.dram_tensor('po', [T, D], BF16, kind='ExternalOutput')

    with nc.allow_low_precision(reason='bf16 matmuls by design'), \
            TileContext(nc) as tc:
        with (
            tc.tile_pool(name='w', bufs=1) as w_pool,
            tc.tile_pool(name='kt', bufs=1) as kt_pool,
            tc.tile_pool(name='vv', bufs=1) as v_pool,
            tc.tile_pool(name='xs', bufs=1) as x_pool,
            tc.tile_pool(name='qt', bufs=2) as qt_pool,
            tc.tile_pool(name='pt', bufs=4) as pt_pool,
            tc.tile_pool(name='ao', bufs=3) as ao_pool,
            tc.tile_pool(name='osb', bufs=2) as osb_pool,
            tc.tile_pool(name='small', bufs=2) as sm_pool,
            tc.tile_pool(name='ps_s', bufs=2, space='PSUM') as ps_s,
            tc.tile_pool(name='ps_ot', bufs=4, space='PSUM') as ps_ot,
        ):
            WK = [w_pool.tile([128, DPC], BF16, tag=f'wk{d}', name=f'wks{d}')
                  for d in range(NDT)]
            WV = [w_pool.tile([128, DPC], BF16, tag=f'wv{d}', name=f'wvs{d}')
                  for d in range(NDT)]
            WQ = [w_pool.tile([128, DPC], BF16, tag=f'wq{d}', name=f'wqs{d}')
                  for d in range(NDT)]
            WO = [w_pool.tile([128, D], BF16, tag=f'wo{d}', name=f'wos{d}')
                  for d in range(4)]
            XS = [x_pool.tile([128, T], BF16, tag=f'x{d}', name=f'xs{d}')
                  for d in range(NDT)]
            KT = [[kt_pool.tile([128, CH], BF16, tag=f'kt{c}_{j}',
                                name=f'kt{c}_{j}') for j in range(4)]
                  for c in range(NC)]
            V = [v_pool.tile([128, VW], BF16, tag=f'v{t}', name=f'v{t}')
                 for t in range(NKB)]

            # pre-warm the ACT exp table and the gpsimd library so the
            # first real exp / affine_select doesn't pay the load
            warm = sm_pool.tile([1, 16], F32, tag='warm', bufs=1)
            warm2 = sm_pool.tile([2, 16], F32, tag='warm2', bufs=1)
            nc.vector.memset(warm[:, :], 0.0)
            nc.scalar.activation(warm[:, :], warm[:, :], AF.Exp)
            nc.gpsimd.affine_select(
                out=warm[:, :], in_=warm[:, :], compare_op=ALU.is_ge,
                fill=0.0, base=0, channel_multiplier=-1, pattern=[[1, 16]])
            nc.gpsimd.partition_broadcast(warm2[:, :], warm[:, :])

            # DMAs ordered by first use: WK and the first x chunk feed the
            # first proj group; WQ next so attention(0,0) S/exp work can
            # fill the PE while the V projection still waits on WV
            for d in range(NDT):
                nc.sync.dma_start(WK[d][:, :], wk_d[d*128:(d+1)*128, :])
                nc.sync.dma_start(XS[d][:, 0:CH],
                                  xT_d[d*128:(d+1)*128, 0:CH])
            for d in range(NDT):
                nc.sync.dma_start(WQ[d][:, :], wq_d[d*128:(d+1)*128, :])
            for d in range(NDT):
                nc.sync.dma_start(XS[d][:, CH:T//2],
                                  xT_d[d*128:(d+1)*128, CH:T//2])
            for d in range(NDT):
                nc.sync.dma_start(WV[d][:, :], wv_d[d*128:(d+1)*128, :])
            for d in range(NDT):
                nc.sync.dma_start(XS[d][:, T//2:T],
                                  xT_d[d*128:(d+1)*128, T//2:T])
            for d in range(4):
                nc.sync.dma_start(WO[d][:, :], wo_d[d*128:(d+1)*128, :])
            # ones columns for the softmax-denominator trick + zero pad
            for t in range(NKB):
                vt3 = V[t][:, 0:HPC*(HD+1)].rearrange(
                    'p (h c) -> p h c', c=HD + 1)
                nc.gpsimd.memset(vt3[:, :, HD], 1.0)
                nc.gpsimd.memset(V[t][:, HPC*(HD+1):VW], 0.0)

            qt_tiles = {}
            ao_tiles = {}

            def proj_group(kind, c, i):
                """One 8-matmul projection group for token chunk c."""
                tok = slice(c*CH, (c+1)*CH)
                if kind == 'K':     # KT[c][i]: out [128 dout, CH tok]
                    pp = ps_ot.tile([128, CH], F32, tag='ot', name='pp')
                    for d in range(NDT):
                        nc.tensor.matmul(
                            pp[:, :], lhsT=WK[d][:, i*128:(i+1)*128],
                            rhs=XS[d][:, tok],
                            start=(d == 0), stop=(d == NDT - 1))
                    nc.vector.tensor_copy(KT[c][i][:, :], pp[:, :])
                elif kind == 'V':   # V block c*4+i: out [128 tok, DPC dout]
                    pv = ps_ot.tile([128, DPC], F32, tag='ot', name='pv')
                    for d in range(NDT):
                        nc.tensor.matmul(
                            pv[:, :],
                            lhsT=XS[d][:, c*CH+i*128:c*CH+(i+1)*128],
                            rhs=WV[d][:, :],
                            start=(d == 0), stop=(d == NDT - 1))
                    vt3 = V[c*4 + i][:, 0:HPC*(HD+1)].rearrange(
                        'p (h c) -> p h c', c=HD + 1)
                    nc.vector.tensor_copy(
                        vt3[:, :, 0:HD],
                        pv.rearrange('p (h c) -> p h c', c=HD))
                elif kind == 'Q':   # QT[c][i]: out [128 dout, CH tok]
                    pq = ps_ot.tile([128, CH], F32, tag='ot', name='pq')
                    for d in range(NDT):
                        nc.tensor.matmul(
                            pq[:, :], lhsT=WQ[d][:, i*128:(i+1)*128],
                            rhs=XS[d][:, tok],
                            start=(d == 0), stop=(d == NDT - 1))
                    qt = qt_pool.tile([128, CH], BF16, tag=f'qt{i}',
                                      name=f'qt{c}_{i}')
                    nc.vector.tensor_copy(qt[:, :], pq[:, :])
                    qt_tiles.setdefault(c, {})[i] = qt

            def outproj_qt(c, qt_i, pj_pool=None, pj_tag='ot'):
                """Out-projection for query rows [c*CH + qt_i*128 ...)."""
                ao = ao_tiles[c]
                q0 = c * CH
                os = osb_pool.tile([128, D], BF16, tag='os', name='os')
                pool = pj_pool or ps_ot
                pjs = [pool.tile([128, 512], F32, tag=pj_tag, name='pj')
                       for _ in range(2)]
                # d outer so each ao weight load serves both halves
                for d in range(4):
                    for half in range(2):
                        nc.tensor.matmul(
                            pjs[half][:, :],
                            lhsT=ao[d][:, qt_i*128:(qt_i+1)*128],
                            rhs=WO[d][:, half*512:(half+1)*512],
                            start=(d == 0), stop=(d == 3))
                for half in range(2):
                    nc.vector.tensor_copy(
                        os[:, half*512:(half+1)*512], pjs[half][:, :])
                nc.sync.dma_start(
                    po_d[q0+qt_i*128:q0+(qt_i+1)*128, :], os[:, :])

            def attention_j(c, j, weave_items=(), tail_items=(), front=False):
                """Causal attention for head pair j over query chunk c.

                weave_items: filler proj/outproj groups emitted between
                k-blocks so TensorE has work during each exp shadow.
                tail_items: outproj groups emitted after the block loop on
                the freed 's' PSUM slots (chunk 3 tail overlap).
                """
                q0 = c * CH
                nkb = (q0 + CH) // 128
                QTj = qt_tiles[c][j]
                h0, h1 = 2*j, 2*j + 1
                ot0 = ps_ot.tile([128, CH], F32, tag='ot', name='ot0')
                ot1 = ps_ot.tile([128, CH], F32, tag='ot', name='ot1')
                witems = list(weave_items)
                boundary = (witems.pop() if len(witems) > 1 and not front
                            else None)
                npair = nkb // 2
                if witems:
                    window = (npair // 2) if front else npair
                    spacing = max(1, window // len(witems))
                else:
                    spacing = 0

                def emit_w(item):
                    kind, wc, wi = item
                    if kind == 'O':
                        outproj_qt(wc, wi)
                    else:
                        proj_group(kind, wc, wi)

                def emit_av(pk, pl, ppt, stop):
                    nc.tensor.matmul(
                        ot0[:, pl:CH],
                        lhsT=V[pk][:, 65*h0:65*h0+128],
                        rhs=ppt[:, pl:CH],
                        start=(pk == 0), stop=stop)
                    nc.tensor.matmul(
                        ot1[:, pl:CH],
                        lhsT=V[pk][:, 65*h1:65*h1+128],
                        rhs=ppt[:, CH+pl:2*CH],
                        start=(pk == 0), stop=stop)

                pend = []
                for kk in range(nkb // 2):
                    # batch S and AV in two-block runs: fewer exposed
                    # LDWEIGHTS at the S<->AV transitions
                    blocks = []
                    for k in (2*kk, 2*kk + 1):
                        lo = max(0, k*128 - q0)
                        s = ps_s.tile([128, 2*CH], F32, tag='s', name='s')
                        pt = pt_pool.tile([128, 2*CH], BF16, tag='pt',
                                          name='pt')
                        ks = KT[k//4][j][:, (k % 4)*128:((k % 4)+1)*128]
                        # S for both heads, concurrent K=64 row tiles
                        nc.tensor.matmul(
                            s[:, lo:CH], lhsT=ks[0:64, :],
                            rhs=QTj[0:64, lo:CH], start=True, stop=True)
                        nc.tensor.matmul(
                            s[:, CH+lo:2*CH], lhsT=ks[64:128, :],
                            rhs=QTj[64:128, lo:CH], start=True, stop=True)
                        blocks.append((k, lo, s, pt))
                    for pk, pl, ppt in pend:
                        emit_av(pk, pl, ppt, stop=False)
                    pend = []
                    for k, lo, s, pt in blocks:
                        if lo:
                            # two-piece AP skips the unwritten columns
                            # [CH:CH+lo] between the head sections
                            s3 = s.rearrange('p (h c) -> p h c', h=2)
                            pt3 = pt.rearrange('p (h c) -> p h c', h=2)
                            nc.scalar.activation(
                                pt3[:, :, lo:CH], s3[:, :, lo:CH], AF.Exp)
                        else:
                            nc.scalar.activation(
                                pt[:, 0:2*CH], s[:, 0:2*CH], AF.Exp)
                        if k*128 >= q0:   # causal mask on diagonal block
                            for c0 in (0, CH):
                                nc.gpsimd.affine_select(
                                    out=pt[:, c0+lo:c0+lo+128],
                                    in_=pt[:, c0+lo:c0+lo+128],
                                    compare_op=ALU.is_ge, fill=0.0,
                                    base=0, channel_multiplier=-1,
                                    pattern=[[1, 128]])
                        pend.append((k, lo, pt))
                    if witems and kk % spacing == spacing - 1:
                        emit_w(witems.pop(0))
                for i, (pk, pl, ppt) in enumerate(pend):
                    emit_av(pk, pl, ppt, stop=(pk == nkb - 1))
                for item in witems:   # leftovers
                    emit_w(item)
                # normalize both heads of the pair
                rp0 = sm_pool.tile([1, CH], F32, tag='rp0', bufs=2)
                rp1 = sm_pool.tile([1, CH], F32, tag='rp1', bufs=2)
                din0 = sm_pool.tile([1, CH], F32, tag='din0', bufs=2)
                din1 = sm_pool.tile([1, CH], F32, tag='din1', bufs=2)
                nc.vector.tensor_copy(din0[:, :], ot0[HD:HD+1, :])
                nc.vector.tensor_copy(din1[:, :], ot1[HD:HD+1, :])
                nc.vector.reciprocal_approx_fast(out=rp0[:, :], in_=din0[:, :])
                nc.vector.reciprocal_approx_fast(out=rp1[:, :], in_=din1[:, :])
                rbs0 = sm_pool.tile([HD, CH], F32, tag='rbs0', bufs=2)
                rbs1 = sm_pool.tile([HD, CH], F32, tag='rbs1', bufs=2)
                nc.gpsimd.partition_broadcast(rbs0[:, :], rp0[:, :])
                nc.gpsimd.partition_broadcast(rbs1[:, :], rp1[:, :])
                ao = ao_tiles[c][j]
                nc.vector.tensor_tensor(
                    out=ao[0:HD, :], in0=ot0[0:HD, :], in1=rbs0[:, :],
                    op=ALU.mult)
                nc.vector.tensor_tensor(
                    out=ao[HD:128, :], in0=ot1[0:HD, :], in1=rbs1[:, :],
                    op=ALU.mult)
                if boundary is not None:
                    emit_w(boundary)
                for kind, wc, wi in tail_items:
                    outproj_qt(wc, wi, pj_pool=ps_s, pj_tag='s')

            # ---------------- emission schedule ----------------
            # chunk-0 projections run d-major across 4 concurrent PSUM
            # accumulators so the PE tracks the weight/x DMA arrival
            # instead of waiting for the full transfer per group
            def proj_chunk0(kind):
                pps = [ps_ot.tile([128, CH], F32, tag='ot', name=f'p0{i}')
                       for i in range(4)]
                for d in range(NDT):
                    for i in range(4):
                        if kind == 'K':
                            nc.tensor.matmul(
                                pps[i][:, :],
                                lhsT=WK[d][:, i*128:(i+1)*128],
                                rhs=XS[d][:, 0:CH],
                                start=(d == 0), stop=(d == NDT - 1))
                        elif kind == 'V':
                            nc.tensor.matmul(
                                pps[i][:, :],
                                lhsT=XS[d][:, i*128:(i+1)*128],
                                rhs=WV[d][:, :],
                                start=(d == 0), stop=(d == NDT - 1))
                        else:
                            nc.tensor.matmul(
                                pps[i][:, :],
                                lhsT=WQ[d][:, i*128:(i+1)*128],
                                rhs=XS[d][:, 0:CH],
                                start=(d == 0), stop=(d == NDT - 1))
                for i in range(4):
                    if kind == 'K':
                        nc.vector.tensor_copy(KT[0][i][:, :], pps[i][:, :])
                    elif kind == 'V':
                        vt3 = V[i][:, 0:HPC*(HD+1)].rearrange(
                            'p (h c) -> p h c', c=HD + 1)
                        nc.vector.tensor_copy(
                            vt3[:, :, 0:HD],
                            pps[i].rearrange('p (h c) -> p h c', c=HD))
                    else:
                        qt = qt_pool.tile([128, CH], BF16, tag=f'qt{i}',
                                          name=f'qt0_{i}')
                        nc.vector.tensor_copy(qt[:, :], pps[i][:, :])
                        qt_tiles.setdefault(0, {})[i] = qt

            proj_chunk0('K')
            proj_chunk0('Q')
            proj_chunk0('V')

            # per chunk c: attention(c) woven with filler matmul groups.
            # proj for chunk c+1 goes into chunk c, except V(3) which lands
            # in chunk 3's first head pair (chunk 3 is exp-bound, chunk 2
            # is matmul-bound); outproj(c) goes into chunk c+1; outproj(3)
            # trails chunk 3's last head pair on the freed 's' PSUM slots.
            for c in range(NC):
                ao_tiles[c] = [ao_pool.tile([128, CH], BF16, tag=f'ao{j}',
                                            name=f'ao{c}_{j}')
                               for j in range(4)]
                # round-robin the filler kinds; out-projection filler lands
                # two chunks later, where the schedule is exp-bound rather
                # than matmul-bound
                kinds = []
                if c + 1 < NC:
                    kinds.append([('K', c+1, i) for i in range(4)])
                    if c + 1 < NC - 1:
                        kinds.append([('V', c+1, i) for i in range(4)])
                    kinds.append([('Q', c+1, i) for i in range(4)])
                if c == 2:
                    kinds.append([('O', 0, i) for i in range(4)])
                    kinds.append([('O', 1, i) for i in range(4)])
                weave = [it for grp in zip(*kinds) for it in grp] if kinds \
                    else []
                if c == NC - 1:
                    # NOTE: ('O', c, 0) must be LAST in jweave[3]: it reads
                    # this chunk's ao[3] and may only be emitted after the
                    # final norm (the boundary slot), never in-loop.
                    jweave = {0: [('V', c, i) for i in range(4)],
                              1: [('O', c-1, 0), ('O', c-1, 1)],
                              2: [('O', c-1, 2)],
                              3: [('O', c-1, 3), ('O', c, 0)]}
                    for j in range(4):
                        attention_j(c, j, jweave[j], front=(j == 0),
                                    tail_items=([('O', c, i)
                                                 for i in range(1, 4)]
                                                if j == 3 else ()))
                else:
                    per_j = (len(weave) + 3) // 4
                    for j in range(4):
                        attention_j(c, j, weave[j*per_j:(j+1)*per_j])

    nc.compile()
    return nc


def _get_nc():
    if 'nc' not in _nc_cache:
        _nc_cache['nc'] = _build_nc()
    return _nc_cache['nc']


def kernel(x, w_qkv, w_out, _profile=False):
    import ml_dtypes
    from concourse.bass_utils import run_bass_kernel_spmd

    x = np.asarray(x, dtype=np.float32)
    w_qkv = np.asarray(w_qkv, dtype=np.float32)
    w_out = np.asarray(w_out, dtype=np.float32)

    nc = _get_nc()

    bf16 = ml_dtypes.bfloat16
    scale = np.float32(1.0 / np.sqrt(HD))
    in_maps = []
    for c in range(N_CORES):
        b, hh = c // 2, c % 2
        s, e = hh * DPC, (hh + 1) * DPC
        in_maps.append({
            'xT': np.ascontiguousarray(x[b].T).astype(bf16),
            'wq': np.ascontiguousarray(w_qkv[:, s:e] * scale).astype(bf16),
            'wk': np.ascontiguousarray(w_qkv[:, D+s:D+e]).astype(bf16),
            'wv': np.ascontiguousarray(w_qkv[:, 2*D+s:2*D+e]).astype(bf16),
            'wo': np.ascontiguousarray(w_out[s:e, :]).astype(bf16),
        })

    res = run_bass_kernel_spmd(nc, in_maps, core_ids=list(range(N_CORES)),
                               trace=_profile)
    out = np.empty((B, T, D), np.float32)
    for b in range(B):
        out[b] = (res.results[2*b]['po'].astype(np.float32)
                  + res.results[2*b+1]['po'].astype(np.float32))
    if _profile:
        return out, res
    return out
